# revision 1
# baseline (speedup 1.0000x reference)
"""Trainium2 Bass kernel for CorpusSupportSets RBF tangent-field.

Math per sample row i (dim 768), with one-hot mask selecting dipole k:
    k    = argmax(mask[i])            (exact: dot with iota row)
    s0,s1 = SUPPORT_SETS[k] halves;  a_j = ALPHAS[k,j];  g_j = exp(LOGGAMMA[k,j])
    zz = |z|^2, ss_j = |s_j|^2, t_j = z.s_j, n_j = zz - 2 t_j + ss_j
    m_j = a_j * g_j * exp(-g_j n_j)
    beta = (m0+m1)(zz-1) - m0 t0 - m1 t1
    p    = beta z + m0 s0 + m1 s1     (= -proj/2 of the reference, sign-safe)
    out  = p / |p|

Sharding: data-parallel over batch across 8 cores (2048 rows each).
The one-hot matmuls of the reference are replaced by an exact index
computation + indirect-DMA row gather from a host-concatenated table
[SUPPORT_SETS | ALPHAS | LOGGAMMA] of shape [1000, 1540].
"""
import sys

for _p in ("/opt/trn_rl_repo",):
    if _p not in sys.path:
        sys.path.insert(0, _p)

import numpy as np

import concourse.bass as bass
import concourse.tile as tile
from concourse import mybir
from concourse.bass import IndirectOffsetOnAxis
from concourse.bass_utils import run_bass_kernel_spmd
from concourse.vector_clock import ScopedClock

# ---------------------------------------------------------------------------
# Workaround: this walrus build only accepts ONE semaphore wait per
# instruction; the TileContext exit drain accumulates one wait per live
# semaphore lane.  Split overflow waits onto trailing sync-engine NOPs.
_MAX_WAITS = 1


def _split_waits(nc, inst):
    si = inst.sync_info
    if si is None:
        return
    waits = list(si.on_wait)
    if len(waits) <= _MAX_WAITS:
        return
    inst.sync_info = mybir.SyncInfo(
        on_wait=waits[:_MAX_WAITS], on_update=list(si.on_update)
    )
    for i in range(_MAX_WAITS, len(waits), _MAX_WAITS):
        nop = nc.sync.nop(nofuse=True, hint="drain_wait_overflow")
        nop.ins.sync_info = mybir.SyncInfo(
            on_wait=waits[i : i + _MAX_WAITS], on_update=[]
        )


def _patched_drain_and_barrier(self, tick_clock, wait_clock):
    drain_inst = self.nc.sync.drain()
    wait_clock.add_sem_waits(
        drain_inst.ins, ScopedClock({None: tick_clock.global_clock})
    )
    _split_waits(self.nc, drain_inst.ins)
    self.nc.all_engine_barrier()
    assert self.sems is not None
    popped = self.nc._tile_sem_poison_stack.pop()
    assert popped is self._sem_poison
    self.nc.clear_and_free_semaphores(list(self.sems.allocated().values()))
    self.nc.all_engine_barrier()


_orig_commit = tile.TileContext._commit_instruction


def _patched_commit(self, inst, lazy_reg_writes=True):
    si = getattr(inst, "sync_info", None)
    if (
        si is not None
        and si.on_wait
        and len(si.on_wait) > _MAX_WAITS
        and inst.engine != mybir.EngineType.Unassigned
    ):
        waits = list(si.on_wait)
        inst.sync_info = mybir.SyncInfo(
            on_wait=waits[:_MAX_WAITS], on_update=list(si.on_update)
        )
        for _i, _w in enumerate(waits[_MAX_WAITS:]):
            nop = mybir.InstNoOp(
                name=f"{inst.name}_w{_i}",
                engine=inst.engine,
                sync_info=mybir.SyncInfo(on_wait=[_w], on_update=[]),
                bass_nofuse=True,
            )
            self._add_instruction(nop)
    return _orig_commit(self, inst, lazy_reg_writes)


tile.TileContext._drain_and_barrier = _patched_drain_and_barrier
tile.TileContext._commit_instruction = _patched_commit

# ---------------------------------------------------------------------------
BS, K, DIM = 16384, 1000, 768
NCORES = 8
ROWS = BS // NCORES  # 2048 rows per core
P = 128
NT = ROWS // P  # 16 tiles of 128 rows
GRP = 4  # tiles per group
NG = NT // GRP  # 4 groups
TBL_W = 2 * DIM + 4  # 1540: [s0 | s1 | a0 a1 lg0 lg1]
F32 = mybir.dt.float32
U32 = mybir.dt.uint32


def build_nc(rows=ROWS):
    NT = rows // P
    NG = NT // GRP
    OP = mybir.AluOpType
    AT = mybir.ActivationFunctionType
    BF16 = mybir.dt.bfloat16
    nc = bass.Bass()
    zin = nc.dram_tensor("zin", [rows, DIM], F32, kind="ExternalInput")
    mk = nc.dram_tensor("mk", [rows, K], BF16, kind="ExternalInput")
    tbl = nc.dram_tensor("tbl", [K, TBL_W], F32, kind="ExternalInput")
    out = nc.dram_tensor("out", [rows, DIM], F32, kind="ExternalOutput")

    with tile.TileContext(nc) as tc:
        with (
            tc.tile_pool(name="zp", bufs=3) as zp,
            tc.tile_pool(name="mkp", bufs=2) as mkp,
            tc.tile_pool(name="selp", bufs=3) as selp,
            tc.tile_pool(name="outp", bufs=2) as outp,
            tc.tile_pool(name="scrD", bufs=4, space="PSUM") as scrDp,
            tc.tile_pool(name="wp", bufs=4) as wp,
            tc.tile_pool(name="tiny", bufs=40) as tinyp,
            tc.tile_pool(name="singles", bufs=1) as singles,
        ):
            ss0a = singles.tile([P, NT], F32)
            ss1a = singles.tile([P, NT], F32)
            q0a = singles.tile([P, NT], F32)
            q1a = singles.tile([P, NT], F32)
            pna = singles.tile([P, NT], F32)
            sqa = singles.tile([P, NT], F32)
            ra = singles.tile([P, NT], F32)
            sidea = singles.tile([P, NT, 4], F32)
            mia = singles.tile([P, NT, 8], U32)

            def phase1(g):
                r0, r1 = g * GRP * P, (g + 1) * GRP * P
                c0, c1 = g * GRP, (g + 1) * GRP
                z_g = zp.tile([P, GRP, DIM], F32, name="z_g", tag="z")
                nc.sync.dma_start(
                    out=z_g[:], in_=zin[r0:r1].rearrange("(n p) c -> p n c", p=P)
                )
                mk_g = mkp.tile([P, GRP, K], mybir.dt.bfloat16, name="mk_g", tag="mk")
                nc.sync.dma_start(
                    out=mk_g[:], in_=mk[r0:r1].rearrange("(n p) c -> p n c", p=P)
                )
                # argmax of one-hot mask, gather table rows into one tile
                sel4 = selp.tile([P, GRP, TBL_W], F32, name="sel4", tag="sel")
                for n in range(GRP):
                    j = c0 + n
                    mx = tinyp.tile([P, 8], mybir.dt.bfloat16, name="mx", tag="mx")
                    nc.vector.max(out=mx[:], in_=mk_g[:, n, :])
                    nc.vector.max_index(
                        out=mia[:, j, :], in_max=mx[:], in_values=mk_g[:, n, :]
                    )
                    nc.gpsimd.indirect_dma_start(
                        out=sel4[:, n, :],
                        out_offset=None,
                        in_=tbl[:],
                        in_offset=IndirectOffsetOnAxis(ap=mia[:, j, 0:1], axis=0),
                    )
                # batched w = z + s for both poles (single big DVE ops)
                w0 = wp.tile([P, GRP, DIM], F32, name="w0", tag="w")
                nc.vector.tensor_tensor(
                    out=w0[:], in0=z_g[:], in1=sel4[:, :, :DIM], op=OP.add
                )
                w1 = wp.tile([P, GRP, DIM], F32, name="w1", tag="w")
                nc.vector.tensor_tensor(
                    out=w1[:], in0=z_g[:], in1=sel4[:, :, DIM : 2 * DIM], op=OP.add
                )
                # per-row reductions on ACT (accumulate along free axis)
                for n in range(GRP):
                    j = c0 + n
                    nc.scalar.activation(
                        out=scrDp.tile([P, DIM], F32, name="scrd", tag="scrD")[:],
                        in_=sel4[:, n, :DIM], func=AT.Square,
                        accum_out=ss0a[:, j : j + 1],
                    )
                    nc.scalar.activation(
                        out=scrDp.tile([P, DIM], F32, name="scrd", tag="scrD")[:],
                        in_=sel4[:, n, DIM : 2 * DIM], func=AT.Square,
                        accum_out=ss1a[:, j : j + 1],
                    )
                    nc.scalar.activation(
                        out=scrDp.tile([P, DIM], F32, name="scrd", tag="scrD")[:],
                        in_=w0[:, n, :], func=AT.Square, accum_out=q0a[:, j : j + 1],
                    )
                    nc.scalar.activation(
                        out=scrDp.tile([P, DIM], F32, name="scrd", tag="scrD")[:],
                        in_=w1[:, n, :], func=AT.Square, accum_out=q1a[:, j : j + 1],
                    )
                nc.gpsimd.tensor_copy(
                    out=sidea[:, c0:c1, :], in_=sel4[:, :, 2 * DIM :]
                )

                # per-group small math on [P, GRP] columns
                def _m(qv, ssv, av, lgv, eng):
                    gt = tinyp.tile([P, GRP], F32, name="gt", tag="tiny")
                    nc.scalar.activation(out=gt[:], in_=lgv, func=AT.Exp)
                    d = tinyp.tile([P, GRP], F32, name="d", tag="tiny")
                    eng.tensor_scalar(
                        out=d[:], in0=ssv, scalar1=1.0, scalar2=None, op0=OP.add
                    )
                    t2 = tinyp.tile([P, GRP], F32, name="t2", tag="tiny")
                    eng.tensor_tensor(out=t2[:], in0=qv, in1=d[:], op=OP.subtract)
                    nn = tinyp.tile([P, GRP], F32, name="nn", tag="tiny")
                    eng.tensor_scalar(
                        out=nn[:], in0=d[:], scalar1=2.0, scalar2=None, op0=OP.mult
                    )
                    eng.tensor_tensor(out=nn[:], in0=nn[:], in1=qv, op=OP.subtract)
                    eng.tensor_tensor(out=nn[:], in0=nn[:], in1=gt[:], op=OP.mult)
                    e = tinyp.tile([P, GRP], F32, name="e", tag="tiny")
                    nc.scalar.activation(out=e[:], in_=nn[:], func=AT.Exp, scale=-1.0)
                    m = tinyp.tile([P, GRP], F32, name="m", tag="tiny")
                    eng.tensor_tensor(out=m[:], in0=e[:], in1=gt[:], op=OP.mult)
                    eng.tensor_tensor(out=m[:], in0=m[:], in1=av, op=OP.mult)
                    return m, t2

                m0, t20 = _m(
                    q0a[:, c0:c1], ss0a[:, c0:c1],
                    sidea[:, c0:c1, 0], sidea[:, c0:c1, 2], nc.vector,
                )
                m1, t21 = _m(
                    q1a[:, c0:c1], ss1a[:, c0:c1],
                    sidea[:, c0:c1, 1], sidea[:, c0:c1, 3], nc.gpsimd,
                )
                # beta = -(m0*t20 + m1*t21)/2   (zz == 1)
                h0 = tinyp.tile([P, GRP], F32, name="h0", tag="tiny")
                nc.vector.tensor_tensor(out=h0[:], in0=m0[:], in1=t20[:], op=OP.mult)
                h1 = tinyp.tile([P, GRP], F32, name="h1", tag="tiny")
                nc.gpsimd.tensor_tensor(out=h1[:], in0=m1[:], in1=t21[:], op=OP.mult)
                bB = tinyp.tile([P, GRP], F32, name="bB", tag="tiny")
                nc.vector.tensor_tensor(out=bB[:], in0=h0[:], in1=h1[:], op=OP.add)
                nc.vector.tensor_scalar(
                    out=bB[:], in0=bB[:], scalar1=-0.5, scalar2=None, op0=OP.mult
                )
                return dict(g=g, z_g=z_g, sel4=sel4, m0=m0, m1=m1, bB=bB)

            def phase2(st):
                g = st["g"]
                r0, r1 = g * GRP * P, (g + 1) * GRP * P
                c0, c1 = g * GRP, (g + 1) * GRP
                z_g, sel4, m0, m1, bB = (
                    st["z_g"], st["sel4"], st["m0"], st["m1"], st["bB"]
                )
                pg = outp.tile([P, GRP, DIM], F32, name="pg", tag="pg")
                for n in range(GRP):
                    j = c0 + n
                    p_n = pg[:, n, :]
                    nc.vector.tensor_scalar(
                        out=p_n, in0=z_g[:, n, :], scalar1=bB[:, n : n + 1],
                        scalar2=None, op0=OP.mult,
                    )
                    nc.vector.scalar_tensor_tensor(
                        out=p_n, in0=sel4[:, n, :DIM], scalar=m0[:, n : n + 1],
                        in1=p_n, op0=OP.mult, op1=OP.add,
                    )
                    nc.vector.scalar_tensor_tensor(
                        out=p_n, in0=sel4[:, n, DIM : 2 * DIM],
                        scalar=m1[:, n : n + 1],
                        in1=p_n, op0=OP.mult, op1=OP.add,
                    )
                    nc.scalar.activation(
                        out=scrDp.tile([P, DIM], F32, name="scrd", tag="scrD")[:],
                        in_=p_n, func=AT.Square, accum_out=pna[:, j : j + 1],
                    )
                nc.scalar.activation(
                    out=sqa[:, c0:c1], in_=pna[:, c0:c1], func=AT.Sqrt
                )
                nc.vector.reciprocal(out=ra[:, c0:c1], in_=sqa[:, c0:c1])
                for n in range(GRP):
                    j = c0 + n
                    nc.vector.tensor_scalar(
                        out=pg[:, n, :], in0=pg[:, n, :], scalar1=ra[:, j : j + 1],
                        scalar2=None, op0=OP.mult,
                    )
                nc.sync.dma_start(
                    out=out[r0:r1].rearrange("(n p) c -> p n c", p=P), in_=pg[:]
                )

            pending = None
            for g in range(NG):
                st = phase1(g)
                if pending is not None:
                    phase2(pending)
                pending = st
            phase2(pending)
    return nc


_NC_CACHE = None


def _get_nc():
    global _NC_CACHE
    if _NC_CACHE is None:
        _NC_CACHE = build_nc()
    return _NC_CACHE


def build_in_maps(inputs):
    import ml_dtypes

    z = np.ascontiguousarray(inputs["z"], dtype=np.float32)
    mask = np.asarray(inputs["support_sets_mask"], dtype=np.float32)
    mk = mask.astype(ml_dtypes.bfloat16)
    tbl = np.ascontiguousarray(
        np.concatenate(
            [
                np.asarray(inputs["SUPPORT_SETS"], dtype=np.float32),
                np.asarray(inputs["ALPHAS"], dtype=np.float32),
                np.asarray(inputs["LOGGAMMA"], dtype=np.float32),
            ],
            axis=1,
        )
    )
    return [
        {
            "zin": np.ascontiguousarray(z[c * ROWS : (c + 1) * ROWS]),
            "mk": np.ascontiguousarray(mk[c * ROWS : (c + 1) * ROWS]),
            "tbl": tbl,
        }
        for c in range(NCORES)
    ]


def kernel(support_sets_mask, z, SUPPORT_SETS, ALPHAS, LOGGAMMA):
    in_maps = build_in_maps(
        dict(
            support_sets_mask=support_sets_mask, z=z,
            SUPPORT_SETS=SUPPORT_SETS, ALPHAS=ALPHAS, LOGGAMMA=LOGGAMMA,
        )
    )
    nc = _get_nc()
    res = run_bass_kernel_spmd(nc, in_maps, list(range(NCORES)))
    return np.concatenate([res.results[c]["out"] for c in range(NCORES)], axis=0)



# revision 7
# speedup vs baseline: 1.5778x; 1.5778x over previous
"""Trainium2 Bass kernel for CorpusSupportSets RBF tangent-field.

Math per sample row i (dim 768), with one-hot mask selecting dipole k
(z is unit-norm, so zz == 1):
    t_j  = z . s_j
    m_j  = c_j * exp(2 g_j t_j),  c_j = a_j g_j exp(-g_j (1 + ss_j))  (host)
    hn   = -(m0 t0 + m1 t1)                      (= beta)
    p    = hn z + m0 s0 + m1 s1                  (= -grad/2, sign-safe)
    |p|^2 = m0^2 ss0 + m1^2 ss1 + 2 m0 m1 d - hn^2,  d = s0 . s1 (host)
    out  = p / |p|   via folding r = 1/|p| into the three coefficients.

Device work per 128-row tile: one max_index (one-hot -> row index), one
indirect row gather from a host-packed bf16 table
[s0 | s1 | c0 c1 g0 g1 ss0 ss1 d pad], two fused multiply-reduce passes
for t0/t1, and three fused passes assembling the normalized output.

Sharding: data-parallel over batch across 8 cores (2048 rows each).
"""
import sys

for _p in ("/opt/trn_rl_repo",):
    if _p not in sys.path:
        sys.path.insert(0, _p)

import numpy as np

import concourse.bass as bass
import concourse.tile as tile
from concourse import mybir
from concourse.bass import IndirectOffsetOnAxis
from concourse.bass_utils import run_bass_kernel_spmd
from concourse.vector_clock import ScopedClock

# ---------------------------------------------------------------------------
# Workaround: this walrus build only accepts ONE semaphore wait per
# instruction; the TileContext exit drain accumulates one wait per live
# semaphore lane.  Split overflow waits onto trailing sync-engine NOPs.
_MAX_WAITS = 1


def _split_waits(nc, inst):
    si = inst.sync_info
    if si is None:
        return
    waits = list(si.on_wait)
    if len(waits) <= _MAX_WAITS:
        return
    inst.sync_info = mybir.SyncInfo(
        on_wait=waits[:_MAX_WAITS], on_update=list(si.on_update)
    )
    for i in range(_MAX_WAITS, len(waits), _MAX_WAITS):
        nop = nc.sync.nop(nofuse=True, hint="drain_wait_overflow")
        nop.ins.sync_info = mybir.SyncInfo(
            on_wait=waits[i : i + _MAX_WAITS], on_update=[]
        )


def _patched_drain_and_barrier(self, tick_clock, wait_clock):
    drain_inst = self.nc.sync.drain()
    wait_clock.add_sem_waits(
        drain_inst.ins, ScopedClock({None: tick_clock.global_clock})
    )
    _split_waits(self.nc, drain_inst.ins)
    self.nc.all_engine_barrier()
    assert self.sems is not None
    popped = self.nc._tile_sem_poison_stack.pop()
    assert popped is self._sem_poison
    self.nc.clear_and_free_semaphores(list(self.sems.allocated().values()))
    self.nc.all_engine_barrier()


_orig_commit = tile.TileContext._commit_instruction


def _patched_commit(self, inst, lazy_reg_writes=True):
    si = getattr(inst, "sync_info", None)
    if (
        si is not None
        and si.on_wait
        and len(si.on_wait) > _MAX_WAITS
        and inst.engine != mybir.EngineType.Unassigned
    ):
        waits = list(si.on_wait)
        inst.sync_info = mybir.SyncInfo(
            on_wait=waits[:_MAX_WAITS], on_update=list(si.on_update)
        )
        for _i, _w in enumerate(waits[_MAX_WAITS:]):
            nop = mybir.InstNoOp(
                name=f"{inst.name}_w{_i}",
                engine=inst.engine,
                sync_info=mybir.SyncInfo(on_wait=[_w], on_update=[]),
                bass_nofuse=True,
            )
            self._add_instruction(nop)
    return _orig_commit(self, inst, lazy_reg_writes)


tile.TileContext._drain_and_barrier = _patched_drain_and_barrier
tile.TileContext._commit_instruction = _patched_commit

# ---------------------------------------------------------------------------
BS, K, DIM = 16384, 1000, 768
NCORES = 8
ROWS = BS // NCORES  # 2048 rows per core
P = 128
NT = ROWS // P  # 16 tiles of 128 rows
GRP = 4  # tiles per group
NG = NT // GRP  # 4 groups
NSIDE = 8  # c0 c1 g0 g1 ss0 ss1 d pad
TBL_W = 2 * DIM + NSIDE  # 1544 bf16 cols
F32 = mybir.dt.float32
F16 = mybir.dt.float16
BF16 = mybir.dt.bfloat16
U8 = mybir.dt.uint8
U32 = mybir.dt.uint32


def build_nc(rows=ROWS):
    NT = rows // P
    NG = NT // GRP
    OP = mybir.AluOpType
    AT = mybir.ActivationFunctionType
    nc = bass.Bass()
    zin = nc.dram_tensor("zin", [rows, DIM], BF16, kind="ExternalInput")
    mk = nc.dram_tensor("mk", [rows, K], U8, kind="ExternalInput")
    tbl = nc.dram_tensor("tbl", [K, TBL_W], BF16, kind="ExternalInput")
    out = nc.dram_tensor("out", [rows, DIM], F16, kind="ExternalOutput")

    with tile.TileContext(nc) as tc:
        with (
            tc.tile_pool(name="zp", bufs=3) as zp,
            tc.tile_pool(name="mkp", bufs=2) as mkp,
            tc.tile_pool(name="selp", bufs=3) as selp,
            tc.tile_pool(name="outp", bufs=2) as outp,
            tc.tile_pool(name="scrp", bufs=4) as scrp,
            tc.tile_pool(name="tiny", bufs=40) as tinyp,
            tc.tile_pool(name="singles", bufs=1) as singles,
        ):
            t0a = singles.tile([P, NT], F32)
            t1a = singles.tile([P, NT], F32)
            m0a = singles.tile([P, NT], F32)
            m1a = singles.tile([P, NT], F32)
            hna = singles.tile([P, NT], F32)
            pna = singles.tile([P, NT], F32)
            sqa = singles.tile([P, NT], F32)
            ra = singles.tile([P, NT], F32)
            rha = singles.tile([P, NT], F32)
            rm0a = singles.tile([P, NT], F32)
            rm1a = singles.tile([P, NT], F32)
            sidef = singles.tile([P, NT, NSIDE], F32)
            mia = singles.tile([P, NT, 8], U32)
            ones8 = singles.tile([P, 8], U8)
            nc.vector.memset(ones8[:], 1)

            def phase1(g):
                r0, r1 = g * GRP * P, (g + 1) * GRP * P
                c0, c1 = g * GRP, (g + 1) * GRP
                z_g = zp.tile([P, GRP, DIM], BF16, name="z_g", tag="z")
                nc.sync.dma_start(
                    out=z_g[:], in_=zin[r0:r1].rearrange("(n p) c -> p n c", p=P)
                )
                mk_g = mkp.tile([P, GRP, K], U8, name="mk_g", tag="mk")
                nc.sync.dma_start(
                    out=mk_g[:], in_=mk[r0:r1].rearrange("(n p) c -> p n c", p=P)
                )
                # one-hot -> row index, gather table rows
                sel4 = selp.tile([P, GRP, TBL_W], BF16, name="sel4", tag="sel")
                for n in range(GRP):
                    j = c0 + n
                    nc.vector.max_index(
                        out=mia[:, j, :], in_max=ones8[:], in_values=mk_g[:, n, :]
                    )
                    nc.gpsimd.indirect_dma_start(
                        out=sel4[:, n, :],
                        out_offset=None,
                        in_=tbl[:],
                        in_offset=IndirectOffsetOnAxis(ap=mia[:, j, 0:1], axis=0),
                    )
                # t_j = z . s_j  (fused multiply-reduce, bf16 in / f32 accum)
                for n in range(GRP):
                    j = c0 + n
                    scr0 = scrp.tile([P, DIM], BF16, name="scr0", tag="scr")
                    nc.vector.scalar_tensor_tensor(
                        out=scr0[:], in0=z_g[:, n, :], scalar=1.0,
                        in1=sel4[:, n, :DIM], op0=OP.mult, op1=OP.mult,
                        accum_out=t0a[:, j : j + 1],
                    )
                    scr1 = scrp.tile([P, DIM], BF16, name="scr1", tag="scr")
                    nc.vector.scalar_tensor_tensor(
                        out=scr1[:], in0=z_g[:, n, :], scalar=1.0,
                        in1=sel4[:, n, DIM : 2 * DIM], op0=OP.mult, op1=OP.mult,
                        accum_out=t1a[:, j : j + 1],
                    )
                # upcast side constants once per group: [P, GRP, 8] -> f32
                nc.scalar.activation(
                    out=sidef[:, c0:c1, :], in_=sel4[:, :, 2 * DIM :], func=AT.Copy
                )
                cs0 = sidef[:, c0:c1, 0]
                cs1 = sidef[:, c0:c1, 1]
                gs0 = sidef[:, c0:c1, 2]
                gs1 = sidef[:, c0:c1, 3]
                ssa = sidef[:, c0:c1, 4]
                ssb = sidef[:, c0:c1, 5]
                dd = sidef[:, c0:c1, 6]
                t0 = t0a[:, c0:c1]
                t1 = t1a[:, c0:c1]

                def _m(tv, gv, cv, ma, eng):
                    gt = tinyp.tile([P, GRP], F32, name="gt", tag="tiny")
                    eng.tensor_tensor(out=gt[:], in0=tv, in1=gv, op=OP.mult)
                    e = tinyp.tile([P, GRP], F32, name="e", tag="tiny")
                    nc.scalar.activation(out=e[:], in_=gt[:], func=AT.Exp, scale=2.0)
                    eng.tensor_tensor(out=ma, in0=e[:], in1=cv, op=OP.mult)

                _m(t0, gs0, cs0, m0a[:, c0:c1], nc.vector)
                _m(t1, gs1, cs1, m1a[:, c0:c1], nc.gpsimd)
                # hn = -(m0 t0 + m1 t1)
                u0 = tinyp.tile([P, GRP], F32, name="u0", tag="tiny")
                nc.vector.tensor_tensor(out=u0[:], in0=m0a[:, c0:c1], in1=t0, op=OP.mult)
                u1 = tinyp.tile([P, GRP], F32, name="u1", tag="tiny")
                nc.gpsimd.tensor_tensor(out=u1[:], in0=m1a[:, c0:c1], in1=t1, op=OP.mult)
                nc.vector.scalar_tensor_tensor(
                    out=hna[:, c0:c1], in0=u0[:], scalar=-1.0, in1=u1[:],
                    op0=OP.mult, op1=OP.subtract,
                )
                # pn = m0^2 ss0 + m1^2 ss1 + 2 m0 m1 d - hn^2
                v0 = tinyp.tile([P, GRP], F32, name="v0", tag="tiny")
                nc.vector.tensor_tensor(
                    out=v0[:], in0=m0a[:, c0:c1], in1=m0a[:, c0:c1], op=OP.mult
                )
                nc.vector.tensor_tensor(out=v0[:], in0=v0[:], in1=ssa, op=OP.mult)
                v1 = tinyp.tile([P, GRP], F32, name="v1", tag="tiny")
                nc.gpsimd.tensor_tensor(
                    out=v1[:], in0=m1a[:, c0:c1], in1=m1a[:, c0:c1], op=OP.mult
                )
                nc.gpsimd.tensor_tensor(out=v1[:], in0=v1[:], in1=ssb, op=OP.mult)
                mm = tinyp.tile([P, GRP], F32, name="mm", tag="tiny")
                nc.vector.tensor_tensor(
                    out=mm[:], in0=m0a[:, c0:c1], in1=m1a[:, c0:c1], op=OP.mult
                )
                nc.vector.tensor_tensor(out=mm[:], in0=mm[:], in1=dd, op=OP.mult)
                hh = tinyp.tile([P, GRP], F32, name="hh", tag="tiny")
                nc.vector.tensor_tensor(
                    out=hh[:], in0=hna[:, c0:c1], in1=hna[:, c0:c1], op=OP.mult
                )
                w = tinyp.tile([P, GRP], F32, name="w", tag="tiny")
                nc.vector.tensor_tensor(out=w[:], in0=v0[:], in1=v1[:], op=OP.add)
                nc.vector.scalar_tensor_tensor(
                    out=w[:], in0=mm[:], scalar=2.0, in1=w[:], op0=OP.mult, op1=OP.add
                )
                nc.vector.tensor_tensor(
                    out=pna[:, c0:c1], in0=w[:], in1=hh[:], op=OP.subtract
                )
                # r = 1/sqrt(pn); fold into coefficients
                nc.scalar.activation(
                    out=sqa[:, c0:c1], in_=pna[:, c0:c1], func=AT.Sqrt
                )
                nc.vector.reciprocal(out=ra[:, c0:c1], in_=sqa[:, c0:c1])
                nc.vector.tensor_tensor(
                    out=rha[:, c0:c1], in0=ra[:, c0:c1], in1=hna[:, c0:c1], op=OP.mult
                )
                nc.vector.tensor_tensor(
                    out=rm0a[:, c0:c1], in0=ra[:, c0:c1], in1=m0a[:, c0:c1], op=OP.mult
                )
                nc.gpsimd.tensor_tensor(
                    out=rm1a[:, c0:c1], in0=ra[:, c0:c1], in1=m1a[:, c0:c1], op=OP.mult
                )
                return dict(g=g, z_g=z_g, sel4=sel4)

            def phase2(st):
                g = st["g"]
                r0, r1 = g * GRP * P, (g + 1) * GRP * P
                c0 = g * GRP
                z_g, sel4 = st["z_g"], st["sel4"]
                pg = outp.tile([P, GRP, DIM], F16, name="pg", tag="pg")
                for n in range(GRP):
                    j = c0 + n
                    p_n = pg[:, n, :]
                    nc.vector.tensor_scalar(
                        out=p_n, in0=z_g[:, n, :], scalar1=rha[:, j : j + 1],
                        scalar2=None, op0=OP.mult,
                    )
                    nc.vector.scalar_tensor_tensor(
                        out=p_n, in0=sel4[:, n, :DIM], scalar=rm0a[:, j : j + 1],
                        in1=p_n, op0=OP.mult, op1=OP.add,
                    )
                    nc.vector.scalar_tensor_tensor(
                        out=p_n, in0=sel4[:, n, DIM : 2 * DIM],
                        scalar=rm1a[:, j : j + 1],
                        in1=p_n, op0=OP.mult, op1=OP.add,
                    )
                nc.sync.dma_start(
                    out=out[r0:r1].rearrange("(n p) c -> p n c", p=P), in_=pg[:]
                )

            pending = None
            for g in range(NG):
                st = phase1(g)
                if pending is not None:
                    phase2(pending)
                pending = st
            phase2(pending)
    return nc


_NC_CACHE = None


def _get_nc():
    global _NC_CACHE
    if _NC_CACHE is None:
        _NC_CACHE = build_nc()
    return _NC_CACHE


def build_in_maps(inputs):
    import ml_dtypes

    bf16 = ml_dtypes.bfloat16
    z = np.asarray(inputs["z"], dtype=np.float32).astype(bf16)
    mask_u8 = (np.asarray(inputs["support_sets_mask"]) != 0).astype(np.uint8)
    SS = np.asarray(inputs["SUPPORT_SETS"], dtype=np.float32)
    AL = np.asarray(inputs["ALPHAS"], dtype=np.float32)
    LG = np.asarray(inputs["LOGGAMMA"], dtype=np.float32)

    s_bf = SS.astype(bf16)  # device sees bf16 s; derive constants from it
    s0 = s_bf[:, :DIM].astype(np.float32)
    s1 = s_bf[:, DIM:].astype(np.float32)
    g = np.exp(LG)
    ss = np.stack([(s0 * s0).sum(1), (s1 * s1).sum(1)], 1)  # [K,2]
    d = (s0 * s1).sum(1)  # [K]
    c = AL * g * np.exp(-g * (1.0 + ss))  # [K,2]
    side = np.zeros((K, NSIDE), dtype=np.float32)
    side[:, 0] = c[:, 0]
    side[:, 1] = c[:, 1]
    side[:, 2] = g[:, 0]
    side[:, 3] = g[:, 1]
    side[:, 4] = ss[:, 0]
    side[:, 5] = ss[:, 1]
    side[:, 6] = d
    tbl = np.ascontiguousarray(
        np.concatenate([s_bf, side.astype(bf16)], axis=1)
    )
    return [
        {
            "zin": np.ascontiguousarray(z[c_ * ROWS : (c_ + 1) * ROWS]),
            "mk": np.ascontiguousarray(mask_u8[c_ * ROWS : (c_ + 1) * ROWS]),
            "tbl": tbl,
        }
        for c_ in range(NCORES)
    ]


def kernel(support_sets_mask, z, SUPPORT_SETS, ALPHAS, LOGGAMMA):
    in_maps = build_in_maps(
        dict(
            support_sets_mask=support_sets_mask, z=z,
            SUPPORT_SETS=SUPPORT_SETS, ALPHAS=ALPHAS, LOGGAMMA=LOGGAMMA,
        )
    )
    nc = _get_nc()
    res = run_bass_kernel_spmd(nc, in_maps, list(range(NCORES)))
    return np.concatenate(
        [res.results[c]["out"] for c in range(NCORES)], axis=0
    ).astype(np.float32)


# revision 16
# speedup vs baseline: 1.7795x; 1.1279x over previous
"""Trainium2 Bass kernel for CorpusSupportSets RBF tangent-field.

Math per sample row i (dim 768), one-hot mask selecting dipole k
(z unit-norm so zz == 1):
    t_j  = z . s_j
    m_j  = c_j exp(2 g_j t_j),   c_j = a_j g_j exp(-g_j (1 + ss_j))   (host)
    hn   = -(m0 t0 + m1 t1)
    pn   = m0^2 ss0 + m1^2 ss1 + 2 m0 m1 d - hn^2,   d = s0 . s1     (host)
    r    = exp(-0.5 ln pn)          (ln+exp share one ACT table set)
    out  = (r hn) z + (r m0) s0 + (r m1) s1          (f16, upcast on host)

Device work per 128-row tile: decode the bit-packed one-hot mask to a row
index (max / max_index / max_index-vs-pow2), indirect-gather one f16 table
row [s0|c0 g0 ss0 .|s1|c1 g1 ss1 d], fused multiply-reduce for t0/t1, and a
3-term normalized assembly with work split across Vector and Scalar engines.

Sharding: data-parallel over batch across 8 cores (2048 rows each).
"""
import sys

for _p in ("/opt/trn_rl_repo",):
    if _p not in sys.path:
        sys.path.insert(0, _p)

import numpy as np

import concourse.bass as bass
import concourse.tile as tile
from concourse import mybir
from concourse.bass import IndirectOffsetOnAxis
from concourse.bass_utils import run_bass_kernel_spmd
from concourse.vector_clock import ScopedClock

# ---------------------------------------------------------------------------
# Workaround: this walrus build only accepts ONE semaphore wait per
# instruction; the TileContext exit drain accumulates one wait per live
# semaphore lane.  Split overflow waits onto trailing sync-engine NOPs.
_MAX_WAITS = 1


def _split_waits(nc, inst):
    si = inst.sync_info
    if si is None:
        return
    waits = list(si.on_wait)
    if len(waits) <= _MAX_WAITS:
        return
    inst.sync_info = mybir.SyncInfo(
        on_wait=waits[:_MAX_WAITS], on_update=list(si.on_update)
    )
    for i in range(_MAX_WAITS, len(waits), _MAX_WAITS):
        nop = nc.sync.nop(nofuse=True, hint="drain_wait_overflow")
        nop.ins.sync_info = mybir.SyncInfo(
            on_wait=waits[i : i + _MAX_WAITS], on_update=[]
        )


def _patched_drain_and_barrier(self, tick_clock, wait_clock):
    drain_inst = self.nc.sync.drain()
    wait_clock.add_sem_waits(
        drain_inst.ins, ScopedClock({None: tick_clock.global_clock})
    )
    _split_waits(self.nc, drain_inst.ins)
    self.nc.all_engine_barrier()
    assert self.sems is not None
    popped = self.nc._tile_sem_poison_stack.pop()
    assert popped is self._sem_poison
    self.nc.clear_and_free_semaphores(list(self.sems.allocated().values()))
    self.nc.all_engine_barrier()


_orig_commit = tile.TileContext._commit_instruction


def _patched_commit(self, inst, lazy_reg_writes=True):
    si = getattr(inst, "sync_info", None)
    if (
        si is not None
        and si.on_wait
        and len(si.on_wait) > _MAX_WAITS
        and inst.engine != mybir.EngineType.Unassigned
    ):
        waits = list(si.on_wait)
        inst.sync_info = mybir.SyncInfo(
            on_wait=waits[:_MAX_WAITS], on_update=list(si.on_update)
        )
        for _i, _w in enumerate(waits[_MAX_WAITS:]):
            nop = mybir.InstNoOp(
                name=f"{inst.name}_w{_i}",
                engine=inst.engine,
                sync_info=mybir.SyncInfo(on_wait=[_w], on_update=[]),
                bass_nofuse=True,
            )
            self._add_instruction(nop)
    return _orig_commit(self, inst, lazy_reg_writes)


tile.TileContext._drain_and_barrier = _patched_drain_and_barrier
tile.TileContext._commit_instruction = _patched_commit

# ---------------------------------------------------------------------------
BS, K, DIM = 16384, 1000, 768
NCORES = 8
ROWS = BS // NCORES  # 2048 rows per core
P = 128
NT = ROWS // P  # 16 tiles of 128 rows
GRP = 4  # tiles per group
NG = NT // GRP  # 4 groups
KB = K // 8  # 125 packed-mask bytes per row
HW_ = DIM + 4  # 772: one pole half [s_j | side_j(4)]
TBL_W = 2 * HW_  # 1544 f16 cols per table row
F32 = mybir.dt.float32
F16 = mybir.dt.float16
U8 = mybir.dt.uint8
U32 = mybir.dt.uint32

# engine-split knobs (tuned from traces)
T_ACT = tuple((j % 8) < 5 for j in range(NT))  # t-reduce on ACT (else DVE stt)
ASM_B = tuple((j % 4) == 0 for j in range(NT))  # asm variant b (else c)


def build_nc(rows=ROWS):
    NT = rows // P
    OP = mybir.AluOpType
    AT = mybir.ActivationFunctionType
    nc = bass.Bass()
    zin = nc.dram_tensor("zin", [rows, DIM], F16, kind="ExternalInput")
    mk = nc.dram_tensor("mk", [rows, KB], U8, kind="ExternalInput")
    tbl = nc.dram_tensor("tbl", [K, TBL_W], F16, kind="ExternalInput")
    pw2in = nc.dram_tensor("pw2", [P, 8], U8, kind="ExternalInput")
    out = nc.dram_tensor("out", [rows, DIM], F16, kind="ExternalOutput")

    with tile.TileContext(nc) as tc:
        with (
            tc.tile_pool(name="zp", bufs=3) as zp,
            tc.tile_pool(name="mkp", bufs=3) as mkp,
            tc.tile_pool(name="selp", bufs=3) as selp,
            tc.tile_pool(name="outp", bufs=2) as outp,
            tc.tile_pool(name="prodp", bufs=4) as prodp,
            tc.tile_pool(name="scrp", bufs=4) as scrp,
            tc.tile_pool(name="psc", bufs=4) as pscp,
            tc.tile_pool(name="tiny", bufs=60) as tinyp,
            tc.tile_pool(name="singles", bufs=1) as singles,
        ):
            t0a = singles.tile([P, NT], F32)
            t1a = singles.tile([P, NT], F32)
            m0a = singles.tile([P, NT], F32)
            m1a = singles.tile([P, NT], F32)
            hna = singles.tile([P, NT], F32)
            pna = singles.tile([P, NT], F32)
            lpa = singles.tile([P, NT], F32)
            ra = singles.tile([P, NT], F32)
            rha = singles.tile([P, NT], F32)
            rm0a = singles.tile([P, NT], F32)
            rm1a = singles.tile([P, NT], F32)
            sidef = singles.tile([P, NT, 2, 4], F32)
            bia = singles.tile([P, NT, 8], U32)
            via = singles.tile([P, NT, 8], U32)
            bfa = singles.tile([P, NT], F32)
            vfa = singles.tile([P, NT], F32)
            ixf = singles.tile([P, NT], F32)
            mia = singles.tile([P, NT], U32)
            pw2 = singles.tile([P, 8], U8)
            nc.sync.dma_start(out=pw2[:], in_=pw2in[:])

            def phase1(g):
                r0, r1 = g * GRP * P, (g + 1) * GRP * P
                c0, c1 = g * GRP, (g + 1) * GRP
                z_g = zp.tile([P, GRP, DIM], F16, name="z_g", tag="z")
                nc.sync.dma_start(
                    out=z_g[:], in_=zin[r0:r1].rearrange("(n p) c -> p n c", p=P)
                )
                mk_g = mkp.tile([P, GRP, KB], U8, name="mk_g", tag="mk")
                nc.sync.dma_start(
                    out=mk_g[:], in_=mk[r0:r1].rearrange("(n p) c -> p n c", p=P)
                )
                # ---- packed one-hot -> byte idx B and bit idx v per tile
                for n in range(GRP):
                    j = c0 + n
                    mx = tinyp.tile([P, 8], U8, name="mx", tag="mx")
                    nc.vector.max(out=mx[:], in_=mk_g[:, n, :])
                    nc.vector.max_index(
                        out=bia[:, j, :], in_max=mx[:], in_values=mk_g[:, n, :]
                    )
                    nc.vector.max_index(
                        out=via[:, j, :], in_max=mx[:], in_values=pw2[:]
                    )
                # idx = 8*B + v  (batched over the group)
                nc.scalar.activation(
                    out=bfa[:, c0:c1], in_=bia[:, c0:c1, 0], func=AT.Copy
                )
                nc.scalar.activation(
                    out=vfa[:, c0:c1], in_=via[:, c0:c1, 0], func=AT.Copy
                )
                nc.vector.scalar_tensor_tensor(
                    out=ixf[:, c0:c1], in0=bfa[:, c0:c1], scalar=8.0,
                    in1=vfa[:, c0:c1], op0=OP.mult, op1=OP.add,
                )
                nc.scalar.activation(
                    out=mia[:, c0:c1], in_=ixf[:, c0:c1], func=AT.Copy
                )
                # ---- gather table rows
                sel4 = selp.tile([P, GRP, TBL_W], F16, name="sel4", tag="sel")
                for n in range(GRP):
                    j = c0 + n
                    nc.gpsimd.indirect_dma_start(
                        out=sel4[:, n, :],
                        out_offset=None,
                        in_=tbl[:],
                        in_offset=IndirectOffsetOnAxis(
                            ap=mia[:, j : j + 1], axis=0
                        ),
                    )
                # ---- t_j = z . s_j
                for n in range(GRP):
                    j = c0 + n
                    if T_ACT[j]:
                        prod = prodp.tile([P, 2, DIM], F16, name="prod", tag="prod")
                        zv = z_g[:, n : n + 1, :].broadcast_to([P, 2, DIM])
                        s2v = sel4[:, n, :].rearrange("p (a w) -> p a w", a=2)[
                            :, :, 0:DIM
                        ]
                        nc.vector.tensor_tensor(
                            out=prod[:], in0=zv, in1=s2v, op=OP.mult
                        )
                        junk0 = scrp.tile([P, DIM], F16, name="junk0", tag="scr")
                        nc.scalar.activation(
                            out=junk0[:], in_=prod[:, 0, :], func=AT.Copy,
                            accum_out=t0a[:, j : j + 1],
                        )
                        junk1 = scrp.tile([P, DIM], F16, name="junk1", tag="scr")
                        nc.scalar.activation(
                            out=junk1[:], in_=prod[:, 1, :], func=AT.Copy,
                            accum_out=t1a[:, j : j + 1],
                        )
                    else:
                        s0cr = scrp.tile([P, DIM], F16, name="s0cr", tag="scr")
                        nc.vector.scalar_tensor_tensor(
                            out=s0cr[:], in0=z_g[:, n, :], scalar=1.0,
                            in1=sel4[:, n, 0:DIM], op0=OP.mult, op1=OP.mult,
                            accum_out=t0a[:, j : j + 1],
                        )
                        s1cr = scrp.tile([P, DIM], F16, name="s1cr", tag="scr")
                        nc.vector.scalar_tensor_tensor(
                            out=s1cr[:], in0=z_g[:, n, :], scalar=1.0,
                            in1=sel4[:, n, HW_ : HW_ + DIM], op0=OP.mult,
                            op1=OP.mult, accum_out=t1a[:, j : j + 1],
                        )
                # upcast side constants [c, g, ss | c, g, ss/d]
                sv = sel4[:].rearrange("p g (a w) -> p g a w", a=2)[:, :, :, DIM:]
                nc.scalar.activation(
                    out=sidef[:, c0:c1, :, :], in_=sv, func=AT.Copy
                )
                return dict(g=g, z_g=z_g, sel4=sel4)

            def smalls(gs):
                """Scalar math for a list of groups (columns c0:c1 each)."""
                OPm, OPa, OPs = OP.mult, OP.add, OP.subtract
                for g in gs:
                    c0, c1 = g * GRP, (g + 1) * GRP
                    cs0 = sidef[:, c0:c1, 0, 0]
                    gs0 = sidef[:, c0:c1, 0, 1]
                    ssa = sidef[:, c0:c1, 0, 2]
                    cs1 = sidef[:, c0:c1, 1, 0]
                    gs1 = sidef[:, c0:c1, 1, 1]
                    ssb = sidef[:, c0:c1, 1, 2]
                    dd = sidef[:, c0:c1, 1, 3]
                    t0 = t0a[:, c0:c1]
                    t1 = t1a[:, c0:c1]

                    def T(nm):
                        return tinyp.tile([P, GRP], F32, name=nm, tag="tiny")

                    gt0, gt1, e0, e1 = T("gt0"), T("gt1"), T("e0"), T("e1")
                    nc.vector.tensor_tensor(out=gt0[:], in0=t0, in1=gs0, op=OPm)
                    nc.vector.tensor_tensor(out=gt1[:], in0=t1, in1=gs1, op=OPm)
                    nc.scalar.activation(out=e0[:], in_=gt0[:], func=mybir.ActivationFunctionType.Exp, scale=2.0)
                    nc.scalar.activation(out=e1[:], in_=gt1[:], func=mybir.ActivationFunctionType.Exp, scale=2.0)
                    nc.vector.tensor_tensor(out=m0a[:, c0:c1], in0=e0[:], in1=cs0, op=OPm)
                    nc.vector.tensor_tensor(out=m1a[:, c0:c1], in0=e1[:], in1=cs1, op=OPm)
                    u0, u1 = T("u0"), T("u1")
                    nc.vector.tensor_tensor(out=u0[:], in0=m0a[:, c0:c1], in1=t0, op=OPm)
                    nc.vector.tensor_tensor(out=u1[:], in0=m1a[:, c0:c1], in1=t1, op=OPm)
                    nc.vector.scalar_tensor_tensor(
                        out=hna[:, c0:c1], in0=u0[:], scalar=-1.0, in1=u1[:],
                        op0=OPm, op1=OPs,
                    )
                    v0, v1, mm, hh, w = T("v0"), T("v1"), T("mm"), T("hh"), T("w")
                    nc.vector.tensor_tensor(out=v0[:], in0=m0a[:, c0:c1], in1=m0a[:, c0:c1], op=OPm)
                    nc.vector.tensor_tensor(out=v0[:], in0=v0[:], in1=ssa, op=OPm)
                    nc.vector.tensor_tensor(out=v1[:], in0=m1a[:, c0:c1], in1=m1a[:, c0:c1], op=OPm)
                    nc.vector.tensor_tensor(out=v1[:], in0=v1[:], in1=ssb, op=OPm)
                    nc.vector.tensor_tensor(out=mm[:], in0=m0a[:, c0:c1], in1=m1a[:, c0:c1], op=OPm)
                    nc.vector.tensor_tensor(out=mm[:], in0=mm[:], in1=dd, op=OPm)
                    nc.vector.tensor_tensor(out=hh[:], in0=hna[:, c0:c1], in1=hna[:, c0:c1], op=OPm)
                    nc.vector.tensor_tensor(out=w[:], in0=v0[:], in1=v1[:], op=OPa)
                    nc.vector.scalar_tensor_tensor(
                        out=w[:], in0=mm[:], scalar=2.0, in1=w[:], op0=OPm, op1=OPa
                    )
                    nc.vector.tensor_tensor(out=pna[:, c0:c1], in0=w[:], in1=hh[:], op=OPs)
                    # r = exp(-0.5 ln pn)   (same ACT table set as Exp)
                    nc.scalar.activation(out=lpa[:, c0:c1], in_=pna[:, c0:c1], func=mybir.ActivationFunctionType.Ln)
                    nc.scalar.activation(out=ra[:, c0:c1], in_=lpa[:, c0:c1], func=mybir.ActivationFunctionType.Exp, scale=-0.5)
                    nc.vector.tensor_tensor(out=rha[:, c0:c1], in0=ra[:, c0:c1], in1=hna[:, c0:c1], op=OPm)
                    nc.vector.tensor_tensor(out=rm0a[:, c0:c1], in0=ra[:, c0:c1], in1=m0a[:, c0:c1], op=OPm)
                    nc.vector.tensor_tensor(out=rm1a[:, c0:c1], in0=ra[:, c0:c1], in1=m1a[:, c0:c1], op=OPm)

            def phase2(st):
                g = st["g"]
                r0, r1 = g * GRP * P, (g + 1) * GRP * P
                c0 = g * GRP
                z_g, sel4 = st["z_g"], st["sel4"]
                pg = outp.tile([P, GRP, DIM], F16, name="pg", tag="pg")
                for n in range(GRP):
                    j = c0 + n
                    p_n = pg[:, n, :]
                    s0v = sel4[:, n, 0:DIM]
                    s1v = sel4[:, n, HW_ : HW_ + DIM]
                    if ASM_B[j]:
                        # z-term on ACT, both pole terms as DVE stt chains
                        nc.scalar.activation(
                            out=p_n, in_=z_g[:, n, :], func=mybir.ActivationFunctionType.Copy,
                            scale=rha[:, j : j + 1],
                        )
                        nc.vector.scalar_tensor_tensor(
                            out=p_n, in0=s0v, scalar=rm0a[:, j : j + 1], in1=p_n,
                            op0=OP.mult, op1=OP.add,
                        )
                        nc.vector.scalar_tensor_tensor(
                            out=p_n, in0=s1v, scalar=rm1a[:, j : j + 1], in1=p_n,
                            op0=OP.mult, op1=OP.add,
                        )
                    else:
                        # pole terms prescaled on ACT, z-term + adds on DVE
                        q0 = pscp.tile([P, DIM], F16, name="q0", tag="psc")
                        nc.scalar.activation(
                            out=q0[:], in_=s0v, func=mybir.ActivationFunctionType.Copy,
                            scale=rm0a[:, j : j + 1],
                        )
                        q1 = pscp.tile([P, DIM], F16, name="q1", tag="psc")
                        nc.scalar.activation(
                            out=q1[:], in_=s1v, func=mybir.ActivationFunctionType.Copy,
                            scale=rm1a[:, j : j + 1],
                        )
                        nc.vector.tensor_scalar(
                            out=p_n, in0=z_g[:, n, :], scalar1=rha[:, j : j + 1],
                            scalar2=None, op0=OP.mult,
                        )
                        nc.vector.tensor_tensor(out=p_n, in0=p_n, in1=q0[:], op=OP.add)
                        nc.vector.tensor_tensor(out=p_n, in0=p_n, in1=q1[:], op=OP.add)
                nc.sync.dma_start(
                    out=out[r0:r1].rearrange("(n p) c -> p n c", p=P), in_=pg[:]
                )

            # pipeline: p1(0) p1(1) S(0,1) p2(0) p1(2) p2(1) p1(3) S(2,3) p2(2) p2(3)
            st0 = phase1(0)
            st1 = phase1(1)
            smalls([0, 1])
            phase2(st0)
            st2 = phase1(2)
            phase2(st1)
            st3 = phase1(3)
            smalls([2, 3])
            phase2(st2)
            phase2(st3)
    return nc


_NC_CACHE = None


def _get_nc():
    global _NC_CACHE
    if _NC_CACHE is None:
        _NC_CACHE = build_nc()
    return _NC_CACHE


def build_in_maps(inputs):
    z = np.asarray(inputs["z"], dtype=np.float32).astype(np.float16)
    mask = np.asarray(inputs["support_sets_mask"])
    mask_bits = np.packbits(mask != 0, axis=1)  # [BS, 125], big-endian bits
    SS = np.asarray(inputs["SUPPORT_SETS"], dtype=np.float32)
    AL = np.asarray(inputs["ALPHAS"], dtype=np.float32)
    LG = np.asarray(inputs["LOGGAMMA"], dtype=np.float32)

    s_f = SS.astype(np.float16)  # device sees f16 s; constants derive from it
    s0 = s_f[:, :DIM].astype(np.float32)
    s1 = s_f[:, DIM:].astype(np.float32)
    g = np.exp(LG)
    ss0 = (s0 * s0).sum(1)
    ss1 = (s1 * s1).sum(1)
    d = (s0 * s1).sum(1)
    c = AL * g * np.exp(-g * (1.0 + np.stack([ss0, ss1], 1)))
    tbl = np.zeros((K, TBL_W), dtype=np.float16)
    tbl[:, 0:DIM] = s_f[:, :DIM]
    tbl[:, DIM + 0] = c[:, 0]
    tbl[:, DIM + 1] = g[:, 0]
    tbl[:, DIM + 2] = ss0
    tbl[:, HW_ : HW_ + DIM] = s_f[:, DIM:]
    tbl[:, HW_ + DIM + 0] = c[:, 1]
    tbl[:, HW_ + DIM + 1] = g[:, 1]
    tbl[:, HW_ + DIM + 2] = ss1
    tbl[:, HW_ + DIM + 3] = d
    tbl = np.ascontiguousarray(tbl)
    pw2 = np.tile(
        np.array([128, 64, 32, 16, 8, 4, 2, 1], np.uint8), (P, 1)
    )
    return [
        {
            "zin": np.ascontiguousarray(z[c_ * ROWS : (c_ + 1) * ROWS]),
            "mk": np.ascontiguousarray(mask_bits[c_ * ROWS : (c_ + 1) * ROWS]),
            "tbl": tbl,
            "pw2": pw2,
        }
        for c_ in range(NCORES)
    ]


def kernel(support_sets_mask, z, SUPPORT_SETS, ALPHAS, LOGGAMMA):
    in_maps = build_in_maps(
        dict(
            support_sets_mask=support_sets_mask, z=z,
            SUPPORT_SETS=SUPPORT_SETS, ALPHAS=ALPHAS, LOGGAMMA=LOGGAMMA,
        )
    )
    nc = _get_nc()
    res = run_bass_kernel_spmd(nc, in_maps, list(range(NCORES)))
    return np.concatenate(
        [res.results[c]["out"] for c in range(NCORES)], axis=0
    ).astype(np.float32)
